# revision 4
# baseline (speedup 1.0000x reference)
"""Trainium2 Bass kernel for nn_DynamicConvolution.

Reference computation (per batch b, T=4096 timesteps, C=512 channels):
    h  = x @ w_in.T + b_in                    # (T, 2C)
    xg = h[:, :C] * sigmoid(h[:, C:])         # GLU -> (T, C)
    w  = softmax((xg @ w_wt.T + b_wt).reshape(T, H, K), axis=-1)
    out[c, t] = sum_k xg[t+k-3, c] * w[t, h(c), k]    # depthwise dynamic conv
    y  = (out + conv_bias) @ w_out.T + b_out

Sharding: data-parallel over batch B=8 -> one batch element per NeuronCore.
Each core runs an identical program on its slice; no collectives.

v2 dataflow (all matmuls bf16, fp32 accumulation), fully C-major:
  - x is host pre-transposed AND pre-cast to bf16; loaded with plain HWDGE
    DMA in t-chunks so mm1 starts almost immediately.
  - mm1 runs weight-stationary: w_in 128x128 slabs are the PE stationary
    operand, xT streams as the moving operand (N=512).  Output lands
    C-major [c_out, t] in PSUM, accumulated over the 4 c_in slabs with a
    1:2 LDWEIGHTS:MATMUL ratio.  Slabs are ordered a0,g0,a1,g1,... so the
    GLU (a * sigmoid(g)) can drain each pair into xgT (C-major, bf16).
  - The weight projection + softmax run in the C-major [hk, t] domain
    directly off xgT (no transpose needed for this path).
  - xgT is PE-transposed tile-by-tile into xg_tok (token-major), which
    the banded-matmul conv needs as its stationary operand.
  - The dynamic conv is a banded matmul per (h, time-tile) exactly as in
    v1: D[t', t] built by one gpsimd local_scatter per time tile.
  - mm_out also runs weight-stationary: w_out slabs stationary, the
    C-major conv output streams (N=512), accumulating y C-major.
    y is written to DRAM as [C, T] f32 and un-transposed on the host.
"""

import os
import sys

import numpy as np

for _p in ("/opt/trn_rl_repo", os.path.expanduser("~/.axon_site/_ro/trn_rl_repo")):
    if os.path.isdir(_p) and _p not in sys.path:
        sys.path.insert(0, _p)

import concourse.bacc as bacc
import concourse.bass as bass
import concourse.mybir as mybir
import concourse.tile as tile
from concourse.bass_utils import run_bass_kernel_spmd

try:
    import ml_dtypes

    BF16 = np.dtype(ml_dtypes.bfloat16)
except ImportError:  # pragma: no cover
    BF16 = None

T, B, C = 4096, 8, 512
H, K = 8, 7
PAD_L = K // 2
C2 = 2 * C
HK = H * K  # 56
P = 128

F32 = mybir.dt.float32
BF = mybir.dt.bfloat16
I16 = mybir.dt.int16

# Dt tile layout: per h a 136-wide block holding the 134 band columns of one
# 128-timestep tile (columns j <-> t = t0 + j - 3).
MAIN_W = 136
DT_W = H * MAIN_W  # 1088
CW = P + 2 * PAD_L  # 134 band columns per tile


def ts(i, size):
    return slice(i * size, (i + 1) * size)


def host_scatter_idxs():
    """Scatter index table: data element (p, i, h) -> column of the Dt tile.

    data[p, i*8+h] = wsm[t0 + p + i - 3, 7h + 6 - i]; its band column is
    j = p + i (column j of block h covers output time t0 + j - 3).
    """
    p = np.arange(P)[:, None, None]
    i = np.arange(K)[None, :, None]
    h = np.arange(H)[None, None, :]
    idx = MAIN_W * h + p + i
    return np.ascontiguousarray(idx.reshape(P, K * H).astype(np.int16))


def build_nc(t_len=T, with_bias_in=False, with_bias_wt=False, with_bias_out=False,
             with_conv_bias=False, dbg=False):
    """Build the single-core Bass program (shared by all 8 cores)."""
    NT = t_len // P     # 128-token time tiles (32)
    NC = t_len // 512   # 512-token chunks (8)
    NG = t_len // 1024  # 1024-token groups (4)
    TPG = NT // NG      # time tiles per group (8)

    nc = bacc.Bacc()

    from contextlib import ExitStack
    _stack_a = ExitStack()

    def ctx_enter(cm):
        return _stack_a.enter_context(cm)

    def ctx_exit():
        _stack_a.close()

    # xT host layout: [p, q, t] = x[t, 128q+p], bf16
    x_d = nc.declare_dram_parameter("xT", [P, 4, t_len], BF, isOutput=False)
    # w_in C-major slabs: [p, q, s, j] = w_in[128s+j, 128q+p], bf16
    w_in_d = nc.declare_dram_parameter("win_cm", [P, 4, 8, P], BF, isOutput=False)
    w_wtT_d = nc.declare_dram_parameter("w_wtT", [P, 4, HK], BF, isOutput=False)
    # w_out C-major slabs: [p, q, co, j] = w_out[128co+j, 128q+p], bf16
    w_out_d = nc.declare_dram_parameter("wout_cm", [P, 4, 4, P], BF, isOutput=False)
    idxs_d = nc.declare_dram_parameter("idxs", [P, HK], I16, isOutput=False)
    ident16_d = nc.declare_dram_parameter("ident16", [P, P], BF, isOutput=False)
    sones8_d = nc.declare_dram_parameter("sones8", [HK, H], BF, isOutput=False)
    sones56_d = nc.declare_dram_parameter("sones56", [H, HK], BF, isOutput=False)
    if with_bias_in:
        # C-major: per-partition bias per output slab s: [p, s] = b_in[128s+p]
        b_in_d = nc.declare_dram_parameter("bin_cm", [P, 8], F32, isOutput=False)
    if with_bias_wt:
        b_wt_d = nc.declare_dram_parameter("b_wt", [HK], F32, isOutput=False)
    if with_bias_out:
        b_out_d = nc.declare_dram_parameter("bout_cm", [P, 4], F32, isOutput=False)
    if with_conv_bias:
        cb4_d = nc.declare_dram_parameter("cb4", [P, 4], F32, isOutput=False)
    # y C-major: [c, t]; host transposes back
    y_d = nc.declare_dram_parameter("y", [C, t_len], F32, isOutput=True)
    if dbg:
        xgT_dbg = nc.declare_dram_parameter("xgT_dbg", [P, 4, t_len], BF, isOutput=True)
        xg_dbg = nc.declare_dram_parameter("xg_dbg", [P, NT, C], BF, isOutput=True)
        wsm_dbg = nc.declare_dram_parameter("wsm_dbg", [P, K, NT, H], BF, isOutput=True)
        conv_dbg = nc.declare_dram_parameter("conv_dbg", [P, 4, t_len], BF, isOutput=True)

    with tile.TileContext(nc) as tc:
        with (
            tc.tile_pool(name="const", bufs=1) as const,
            tc.tile_pool(name="big", bufs=1) as big,
            tc.tile_pool(name="work", bufs=3) as work,
            tc.tile_pool(name="glu", bufs=2) as glu,
            tc.tile_pool(name="dtp", bufs=8) as dtp,
            tc.tile_pool(name="outp", bufs=3) as outp,
        ):
            # ---- constants ----
            sb_win = const.tile([P, 4, 8, P], BF)
            nc.sync.dma_start(sb_win[:, :, 0:2, :], w_in_d[:, :, 0:2, :])
            nc.sync.dma_start(sb_win[:, :, 2:8, :], w_in_d[:, :, 2:8, :])
            sb_wwtT = const.tile([P, 4, HK], BF)
            nc.sync.dma_start(sb_wwtT[:], w_wtT_d[:])
            sb_wout = const.tile([P, 4, 4, P], BF)
            nc.sync.dma_start(sb_wout[:], w_out_d[:])
            sb_idxs = const.tile([P, HK], I16)
            nc.sync.dma_start(sb_idxs[:], idxs_d[:])
            sb_id16 = const.tile([P, P], BF)
            nc.sync.dma_start(sb_id16[:], ident16_d[:])
            sb_so8 = const.tile([HK, H], BF)
            nc.sync.dma_start(sb_so8[:], sones8_d[:])
            sb_so56 = const.tile([H, HK], BF)
            nc.sync.dma_start(sb_so56[:], sones56_d[:])
            if with_bias_in:
                sb_bin = const.tile([P, 8], F32)
                nc.sync.dma_start(sb_bin[:], b_in_d[:])
            if with_bias_wt:
                sb_bwt = const.tile([HK, 1], F32)
                nc.sync.dma_start(sb_bwt[:], b_wt_d[:, None])
            if with_bias_out:
                sb_bout = const.tile([P, 4], F32)
                nc.sync.dma_start(sb_bout[:], b_out_d[:])
            if with_conv_bias:
                sb_cb4 = const.tile([P, 4], F32)
                nc.sync.dma_start(sb_cb4[:], cb4_d[:])

            # ---- persistent activations ----
            xT = big.tile([P, 4, t_len], BF)       # [c%128, c//128, t]
            xgT = big.tile([P, 4, t_len], BF)      # [c%128, c//128, t]
            xg = big.tile([P, NT, C], BF)          # [t%128, t//128, c]
            conv = big.tile([P, 4, t_len], BF)     # [c%128, c//128, t]
            wsm3 = big.tile([P, K, NT, H], BF)     # [t%128, k, t//128, h]
            data_tmp = big.tile([P, K, NT, H], BF)
            data_all = big.tile([P, NT, HK], BF)

            # x streamed in 512-column chunks (contiguous [128, 4, 512] blocks)
            for n in range(NC):
                nc.sync.dma_start(xT[:, :, ts(n, 512)], x_d[:, :, ts(n, 512)])

            nc.gpsimd.memset(data_tmp[:], 0.0)

            ps_mm1 = ctx_enter(tc.tile_pool(name="ps_mm1", bufs=1,
                                            space=bass.MemorySpace.PSUM))
            ps_tr = ctx_enter(tc.tile_pool(name="ps_tr", bufs=2,
                                           space=bass.MemorySpace.PSUM))
            ps_wl = ctx_enter(tc.tile_pool(name="ps_wl", bufs=1,
                                           space=bass.MemorySpace.PSUM))
            ps_ss = ctx_enter(tc.tile_pool(name="ps_ss", bufs=1,
                                           space=bass.MemorySpace.PSUM))

            # ======== pass 1a: mm1 weight-stationary -> GLU -> xgT ========
            # Per 1024-token group g and c_out pair (a_i, g_i): accumulate
            # [128, 2, 512] PSUM tiles over the 4 c_in slabs, then GLU.
            def mm1_group(g):
                tg = slice(g * 1024, (g + 1) * 1024)
                for i in range(4):      # c_out slab pair index
                    ps_a = ps_mm1.tile([P, 2, 512], F32, tag="ps_a")
                    ps_g = ps_mm1.tile([P, 2, 512], F32, tag="ps_g")
                    for q in range(4):
                        wa = sb_win[:, q, i, :]
                        wg = sb_win[:, q, 4 + i, :]
                        for tci in range(2):
                            mv = xT[:, q, ts(2 * g + tci, 512)]
                            nc.tensor.matmul(ps_a[:, tci, :], wa, mv,
                                             start=(q == 0), stop=(q == 3))
                        for tci in range(2):
                            mv = xT[:, q, ts(2 * g + tci, 512)]
                            nc.tensor.matmul(ps_g[:, tci, :], wg, mv,
                                             start=(q == 0), stop=(q == 3))
                    a_sb = glu.tile([P, 2, 512], BF, tag="a_sb")
                    sig = glu.tile([P, 2, 512], BF, tag="sig")
                    for tci in range(2):
                        if with_bias_in:
                            nc.vector.tensor_scalar_add(
                                a_sb[:, tci, :], ps_a[:, tci, :],
                                sb_bin[:, i:i + 1])
                            nc.scalar.activation(
                                sig[:, tci, :], ps_g[:, tci, :],
                                mybir.ActivationFunctionType.Sigmoid,
                                bias=sb_bin[:, 4 + i:5 + i])
                        else:
                            with nc.allow_low_precision(reason="bf16 act"):
                                nc.vector.tensor_copy(a_sb[:, tci, :],
                                                      ps_a[:, tci, :])
                            nc.scalar.activation(
                                sig[:, tci, :], ps_g[:, tci, :],
                                mybir.ActivationFunctionType.Sigmoid)
                        nc.vector.tensor_mul(
                            xgT[:, i, ts(2 * g + tci, 512)],
                            a_sb[:, tci, :], sig[:, tci, :])

            # ======== pass 1c: dynamic weights + softmax (C-major) ========
            def pass1c_tile(n):
                pw2 = ps_wl.tile([HK, 512], F32, tag="w1")
                for q in range(4):
                    nc.tensor.matmul(pw2[:], sb_wwtT[:, q, :],
                                     xgT[:, q, ts(n, 512)],
                                     start=(q == 0), stop=(q == 3))
                e2 = work.tile([HK, 512], BF, tag="e2")
                if with_bias_wt:
                    nc.scalar.activation(e2[:], pw2[:],
                                         mybir.ActivationFunctionType.Exp,
                                         bias=sb_bwt[:])
                else:
                    nc.scalar.activation(e2[:], pw2[:],
                                         mybir.ActivationFunctionType.Exp)
                ps_s = ps_ss.tile([HK, 512], F32, tag="ss")
                nc.tensor.matmul(ps_s[0:H, :], sb_so8[:], e2[:], start=True,
                                 stop=True)
                r8f = work.tile([H, 512], F32, tag="r8f")
                nc.vector.reciprocal_approx_fast(r8f[:], ps_s[0:H, :])
                r8 = work.tile([H, 512], BF, tag="r8")
                with nc.allow_low_precision(reason="softmax 1/s in bf16 is fine"):
                    nc.vector.tensor_copy(r8[:], r8f[:])
                ps_r = ps_ss.tile([HK, 512], F32, tag="ss")
                nc.tensor.matmul(ps_r[:], sb_so56[:], r8[:], start=True, stop=True)
                wsmC = work.tile([HK, 512], BF, tag="wsmC")
                nc.vector.tensor_mul(wsmC[:], e2[:], ps_r[:])
                # back to token-major: wsm3[p, k, m, h] = w_sm[128m+p, 7h+k]
                ptr = ps_wl.tile([P, 4, HK], BF, tag="w1")
                for j in range(4):
                    nc.tensor.transpose(ptr[:, j, :], wsmC[:, ts(j, P)],
                                        sb_id16[0:HK, 0:HK])
                w_dst = wsm3[:, :, ts(n, 4), :].transpose([0, 2, 3, 1])
                nc.vector.tensor_copy(
                    w_dst, ptr[:].rearrange("p m (h k) -> p m h k", k=K))

            # ======== pass 1t: xgT -> xg (token-major) via PE transpose ====
            def transpose_tile(m):
                pxg = ps_tr.tile([P, 4, P], BF, tag="pxg")
                for q in range(4):
                    nc.tensor.transpose(pxg[:, q, :], xgT[:, q, ts(m, P)],
                                        sb_id16[:])
                if m % 2 == 0:
                    nc.scalar.copy(xg[:, m, :], pxg[:])
                else:
                    with nc.allow_low_precision(reason="bf16 copy"):
                        nc.vector.tensor_copy(xg[:, m, :], pxg[:])

            def build_group(mlo, mhi):
                # shifted copies of wsm3 feeding the band scatter
                for i in range(K):
                    d = i - 3
                    kk = 6 - i
                    if d == 0:
                        nc.sync.dma_start(data_tmp[:, i, mlo:mhi, :],
                                          wsm3[:, kk, mlo:mhi, :])
                    elif d < 0:
                        nc.sync.dma_start(data_tmp[-d:P, i, mlo:mhi, :],
                                          wsm3[0:P + d, kk, mlo:mhi, :])
                        lo = max(mlo, 1)
                        if lo < mhi:
                            nc.sync.dma_start(data_tmp[0:-d, i, lo:mhi, :],
                                              wsm3[P + d:P, kk, lo - 1:mhi - 1, :])
                    else:
                        nc.sync.dma_start(data_tmp[0:P - d, i, mlo:mhi, :],
                                          wsm3[d:P, kk, mlo:mhi, :])
                        hi = min(mhi, NT - 1)
                        if mlo < hi:
                            nc.sync.dma_start(data_tmp[P - d:P, i, mlo:hi, :],
                                              wsm3[0:d, kk, mlo + 1:hi + 1, :])
                # permute [p, i, m, h] -> [p, m, (i, h)]
                da4 = data_all[:, mlo:mhi, :].rearrange("p m (i h) -> p m i h", h=H)
                nc.vector.tensor_copy(
                    da4, data_tmp[:, :, mlo:mhi, :].transpose([0, 2, 1, 3]))

            for g in range(NG):
                mm1_group(g)
                pass1c_tile(2 * g)
                pass1c_tile(2 * g + 1)
                for m in range(g * TPG, (g + 1) * TPG):
                    transpose_tile(m)
                if g >= 1:
                    build_group((g - 1) * TPG, g * TPG)
            build_group((NG - 1) * TPG, NT)

            if dbg:
                nc.sync.dma_start(xgT_dbg[:], xgT[:])
                nc.sync.dma_start(xg_dbg[:], xg[:])
                nc.sync.dma_start(wsm_dbg[:], wsm3[:])

            # ======== pass 2: banded-matmul conv + weight-stationary out ====
            ctx_exit()  # release pass-1 PSUM pools
            ps_c = ctx_enter(tc.tile_pool(name="ps_c", bufs=2,
                                          space=bass.MemorySpace.PSUM))
            ps_o = ctx_enter(tc.tile_pool(name="ps_o", bufs=2,
                                          space=bass.MemorySpace.PSUM))

            def conv_matmuls(m):
                dt = dtp.tile([P, DT_W], BF, tag="dt")
                nc.gpsimd.local_scatter(dt[:], data_all[:, m, :], sb_idxs[:],
                                        channels=P, num_elems=DT_W, num_idxs=HK)
                # [128, 4, 256] f32 = two PSUM banks; each 134-wide plane pair
                # stays inside a single bank
                pc = ps_c.tile([P, 4, 256], F32, tag="pc")
                pc = pc[:, :, 0:CW]
                for ci in range(4):
                    for hp, pb in ((0, 0), (1, 64)):
                        hh = ci * 2 + hp
                        nc.tensor.matmul(
                            pc[pb:pb + 64, ci, :], xg[:, m, ts(hh, 64)],
                            dt[:, MAIN_W * hh:MAIN_W * hh + CW],
                            start=True, stop=True, skip_group_check=True)
                return pc

            def mm_out_group(g):
                # y[c_out, t] for t in [1024g, 1024g+1024): accumulate over
                # the 4 c_in slabs of conv, w_out stationary.
                for co in range(4):
                    py = ps_o.tile([P, 2, 512], F32, tag="py")
                    for q in range(4):
                        wslab = sb_wout[:, q, co, :]
                        for tci in range(2):
                            mv = conv[:, q, ts(2 * g + tci, 512)]
                            nc.tensor.matmul(py[:, tci, :], wslab, mv,
                                             start=(q == 0), stop=(q == 3))
                    out_t = outp.tile([P, 2, 512], F32, tag="out_t")
                    for tci in range(2):
                        if with_bias_out:
                            nc.vector.tensor_scalar_add(
                                out_t[:, tci, :], py[:, tci, :],
                                sb_bout[:, co:co + 1])
                        else:
                            nc.vector.tensor_copy(out_t[:, tci, :],
                                                  py[:, tci, :])
                    nc.sync.dma_start(y_d[ts(co, P), ts(g, 1024)],
                                      out_t[:].rearrange("p a b -> p (a b)"))

            el_prev = None
            for m in range(NT):
                pc_m = conv_matmuls(m)
                t0 = m * P
                # body of tile m (must precede the left-edge add)
                if with_conv_bias:
                    for ci in range(4):
                        nc.vector.tensor_scalar_add(
                            conv[:, ci, t0:t0 + P], pc_m[:, ci, PAD_L:PAD_L + P],
                            sb_cb4[:, ci:ci + 1])
                else:
                    if m % 2 == 0:
                        nc.scalar.copy(conv[:, :, t0:t0 + P],
                                       pc_m[:, :, PAD_L:PAD_L + P])
                    else:
                        with nc.allow_low_precision(reason="bf16 conv body"):
                            nc.vector.tensor_copy(conv[:, :, t0:t0 + P],
                                                  pc_m[:, :, PAD_L:PAD_L + P])
                if el_prev is not None:
                    # left edge of tile m: slab m-1 rows feeding t0..t0+2
                    dl = conv[:, :, t0:t0 + PAD_L]
                    nc.vector.tensor_add(dl, dl, el_prev[:])
                    # right edge of tile m-1: slab m rows feeding its tail
                    dr = conv[:, :, t0 - PAD_L:t0]
                    nc.vector.tensor_add(dr, dr, pc_m[:, :, 0:PAD_L])
                if m + 1 < NT:
                    # stage the outgoing right-edge so pc needs one generation
                    el = work.tile([P, 4, PAD_L], F32, tag="el")
                    nc.vector.tensor_copy(el[:], pc_m[:, :, CW - PAD_L:CW])
                    el_prev = el
                # group g's conv tiles are edge-complete once tile 8g's left
                # edge has been applied -> emit the previous group's mm_out
                if m % TPG == 0 and m >= TPG:
                    mm_out_group(m // TPG - 1)
            mm_out_group(NG - 1)

            if dbg:
                nc.sync.dma_start(conv_dbg[:], conv[:])

            ctx_exit()  # release pass-2 PSUM pools

    nc.compile()
    return nc


def host_inputs(x_b, w_in, b_in, w_wt, b_wt, w_out, b_out, conv_bias,
                with_bias_in, with_bias_wt, with_bias_out, with_conv_bias):
    """Per-core input map from a batch slice + shared weights."""
    hk_of = np.arange(HK) // K

    def cm_pack(w, n_out_slabs):
        # w: [n_out_slabs*128, C] -> [128, 4, n_out_slabs, 128]
        # [p, q, s, j] = w[128s+j, 128q+p]
        a = np.asarray(w, np.float32).reshape(n_out_slabs, P, 4, P)
        return np.ascontiguousarray(a.transpose(3, 2, 0, 1)).astype(BF16)

    def t_pack(w, width):
        # w: [width, C] -> [128, 4, width] with [p, q, f] = w[f, 128q+p]
        return np.ascontiguousarray(
            np.asarray(w, np.float32).T.reshape(4, P, width)
            .transpose(1, 0, 2)).astype(BF16)

    # xT [p, q, t] = x[t, 128q+p]
    xT = np.ascontiguousarray(
        np.asarray(x_b, np.float32).T.reshape(4, P, T).transpose(1, 0, 2)
    ).astype(BF16)

    m = {
        "xT": xT,
        "win_cm": cm_pack(w_in, 8),
        "w_wtT": t_pack(w_wt, HK),
        "wout_cm": cm_pack(w_out, 4),
        "idxs": host_scatter_idxs(),
        "ident16": np.eye(P).astype(BF16),
        "sones8": (hk_of[:, None] == np.arange(H)[None, :]).astype(BF16),
        "sones56": (np.arange(H)[:, None] == hk_of[None, :]).astype(BF16),
    }
    if with_bias_in:
        m["bin_cm"] = np.ascontiguousarray(
            np.asarray(b_in, np.float32).reshape(8, P).T)
    if with_bias_wt:
        m["b_wt"] = np.asarray(b_wt, np.float32)
    if with_bias_out:
        m["bout_cm"] = np.ascontiguousarray(
            np.asarray(b_out, np.float32).reshape(4, P).T)
    if with_conv_bias:
        m["cb4"] = np.ascontiguousarray(
            np.asarray(conv_bias, np.float32).reshape(4, P).T)
    return m


_NC_CACHE = {}


def _get_nc(key):
    if key not in _NC_CACHE:
        _NC_CACHE[key] = build_nc(T, *key)
    return _NC_CACHE[key]


def kernel(x, w_in, b_in, w_wt, b_wt, w_out, b_out, conv_bias, _trace=False):
    x = np.asarray(x)
    flags = (bool(np.any(b_in)), bool(np.any(b_wt)), bool(np.any(b_out)),
             bool(np.any(conv_bias)))
    nc = _get_nc(flags)
    in_maps = [
        host_inputs(x[:, b, :], np.asarray(w_in), b_in, np.asarray(w_wt), b_wt,
                    np.asarray(w_out), b_out, conv_bias, *flags)
        for b in range(B)
    ]
    res = run_bass_kernel_spmd(nc, in_maps, core_ids=list(range(B)),
                               trace=_trace)
    # y comes back C-major [C, T]; un-transpose to (T, B, C)
    y = np.stack([np.asarray(res.results[b]["y"]).T for b in range(B)], axis=1)
    if _trace:
        return y.astype(np.float32), res
    return y.astype(np.float32)
